# revision 14
# baseline (speedup 1.0000x reference)
"""Trainium2 Bass kernel for nn_NeuronS3DiffUpsample2D.

Reference computation (per sample b):
    up   = nearest-2x-upsample(x[b])                       # [C, 320, 320]
    w    = Wb + 0.25 * einsum('or,rikl->oikl', lora_up, lora_down)
    w_b  = w * de_mod[b, None, :, None, None]              # modulate input chans
    dem  = rsqrt(sum_{i,k,l} w_b^2 + eps)                  # per output chan
    y[b] = conv2d(up, w_b * dem, SAME) + bias

Key algebraic transform: a 3x3 SAME conv on a 2x nearest-upsampled image
decomposes into 4 output phases (di, dj in {0,1}), each a 2x2 conv on the
ORIGINAL 160x160 input:
    y[2i+di, 2j+dj] = sum_{a,b in {0,1}} K[di,dj,a,b] @ x[i+a+di-1, j+b+dj-1]
where the 16 [O, I] matrices K are sums of 1/2/4 of the 9 taps of w.
This is 4/9 of the naive FLOPs and never materializes the upsampled image.

Since the demod scale is per output channel and conv is linear in w, the conv
OUTPUT is scaled by dem[o] (per-partition scalar) at PSUM eviction, fused with
the bias add; weights are only modulated by de_mod on the input-channel axis.

Sharding: data-parallel over batch B=8 across 8 NeuronCores; each core builds
its own per-sample weights locally (replicated W/lora are tiny).  Host-side
work is layout only: per-sample slicing, weight transposition, bf16 rounding,
pre-haloed banding of x, and the final phase interleave of the output.

Everything on the wire is bf16 (error budget is 2e-2; bf16 end-to-end lands
around 3e-3): x is DMA'd as pre-haloed zero-padded bands (one contiguous
descriptor per partition), weights/comb are bf16, and the output is written
phase-separated bf16 (contiguous evictions, half the HBM writes); the host
interleaves the 4 phases into y and upcasts to f32 - pure layout.  PSUM
accumulation and the demod chain stay f32.

Scheduling: the PE is in-order, so the startup is arranged to unblock the
conv stream as early as possible: weight DMAs are queued before the band
DMAs, band 0 is split so its first half lands sooner, the modulated weight
build is split so phase-0 slots exist before the rest, the 8 copy-slots run
on ACT while the 8 add-slots run on DVE, and the demod partition-sum matmul
is emitted AFTER block 0 so it cannot stall the conv stream (its DVE/ACT
inputs compute in the shadow of block 0's matmuls).
"""

import sys
import numpy as np
from contextlib import ExitStack

try:
    import concourse.bass as bass
except ImportError:  # grading env without the axon PYTHONPATH
    sys.path.insert(0, "/opt/trn_rl_repo")
    import concourse.bass as bass
import ml_dtypes
import concourse.tile as tile
from concourse import bacc, mybir
from concourse.bass_utils import run_bass_kernel_spmd

B, C, H, W = 8, 128, 160, 160
RANK = 32
SCALING = 0.25
EPS = 1e-8
WP = W + 2          # padded row length (zero col on each side)
BAND_ROWS = 27      # x-rows per band
NBANDS = (H + BAND_ROWS - 1) // BAND_ROWS   # 6
BAND_TROWS = BAND_ROWS + 2           # band rows incl. halo (29)
B0SPLIT = 14        # band0 lands as rows [0,14) then [14,29)
NCORES = 8

f32 = mybir.dt.float32
bf16 = mybir.dt.bfloat16
np_bf16 = ml_dtypes.bfloat16


def _blocks():
    """(i0, R, band) row blocks: 3 rows per matmul (N=480), except the last
    band's tail 4 rows split 2+2 so every matmul keeps free dim >= 256."""
    out = []
    for bb in range(NBANDS):
        lo, hi = BAND_ROWS * bb, min(BAND_ROWS * (bb + 1), H)
        i = lo
        while i < hi:
            r = 2 if hi - i == 4 else min(3, hi - i)
            out.append((i, r, bb))
            i += r
    return out

BLOCKS = _blocks()


def _slot(di, a, dj, b):
    return 8 * di + 2 * a + 4 * dj + b


def _conv_kernel(ctx, tc, y, x, dmbias, wbT, lpkD, ident2):
    nc = tc.nc
    AF = mybir.ActivationFunctionType
    ALU = mybir.AluOpType
    AX = mybir.AxisListType

    const = ctx.enter_context(tc.tile_pool(name="const", bufs=1))
    bands = ctx.enter_context(tc.tile_pool(name="bands", bufs=3))

    comb = const.tile([128, 16, C], bf16)        # 16 combined taps, [i, slot, o]
    demP = const.tile([128, 1], f32)             # rsqrt demod, per output chan
    dmb = const.tile([128, 3], f32)              # de_mod[i], bias[o], 0.25*de_mod
    s2b = const.tile([128, C], bf16)             # per-(i,o) weight square sums
    onesS = const.tile([128, 1], bf16)
    t1 = const.tile([128, 1], f32)               # sum w^2 + eps
    sdem = const.tile([128, 1], f32)             # sqrt(sum w^2 + eps)

    # de_mod/bias arrive as a [2,128] row pair (single-descriptor DMA) and are
    # PE-transposed onto partitions.
    dmbR = const.tile([2, C], f32)
    nc.sync.dma_start(dmbR[:], dmbias[:])
    id2 = const.tile([2, 2], f32)
    nc.sync.dma_start(id2[:], ident2[:])

    # PE warm-up: the HAM clock gate keeps the PE at 1.2 GHz until it has
    # seen ~3.4us of sustained matmul activity.  A burst of throwaway
    # matmuls on zeroed SBUF during the (otherwise idle) DMA window flips
    # the gate to 2.4 GHz right as the real stream begins.
    wz = const.tile([128, 512], bf16)
    nc.vector.memset(wz[:], 0.0)

    with tc.tile_pool(name="wtmp", bufs=1) as wtmp, tc.tile_pool(
        name="wpsum", bufs=1, space="PSUM"
    ) as wpsum:
        # ---- input DMAs, all on one SWDGE queue so FIFO order gives the
        # tiny weight tensors strict priority over the 7 MB of band traffic
        # (the SDMA engines round-robin *between* queues at packet
        # granularity, which would starve the weights for ~10us).  Band 0
        # is interleaved right after the lora weights and split in two so
        # the first conv matmuls start as early as possible.
        lpk = wtmp.tile([RANK, 10, C], bf16)     # [lora_up^T | lora_down^T]
        nc.gpsimd.dma_start(lpk[:], lpkD[:])
        LUTn = lpk[:, 0, :]                      # lora_up^T: [r, o]
        LD9 = lpk[:, 1:10, :]                    # lora_down^T: [r, t, i]
        WbTS = wtmp.tile([128, 9, C], bf16)      # Wb^T: [i, t, o]
        nc.gpsimd.dma_start(WbTS[:], wbT[:])
        band_tiles = [
            bands.tile([128, BAND_TROWS, WP], bf16, tag="band", name=f"band{bb}")
            for bb in range(NBANDS)
        ]
        nc.gpsimd.dma_start(band_tiles[0][:, :B0SPLIT, :], x[:, 0, :B0SPLIT, :])
        nc.gpsimd.dma_start(band_tiles[0][:, B0SPLIT:, :], x[:, 0, B0SPLIT:, :])
        for bb in range(1, NBANDS):
            nc.gpsimd.dma_start(band_tiles[bb][:], x[:, bb])

        # 12 x N=512 fills ~4.5us: flips the HAM gate AND spans the input
        # DMAs' ~2us completion latency so the PE never idles before the
        # delta matmuls.
        zpsum = wpsum.tile([128, 512], f32)
        for _ in range(12):
            nc.tensor.matmul(zpsum[:], wz[:, 0:128], wz[:], start=True, stop=True)

        dmbP = wpsum.tile([128, 2], f32)
        nc.tensor.transpose(dmbP[:], dmbR[:], id2[:])
        nc.vector.tensor_copy(dmb[:, 0:2], dmbP[:])
        nc.vector.tensor_scalar_mul(dmb[:, 2:3], dmb[:, 0:1], SCALING)
        nc.vector.memset(onesS[:], 1.0)

        # deltaT_unscaled[i, t, o] = sum_r down[r,i,t] * up[o,r]; t=0..2 first
        # so the wm3 front half can start after only 3 matmuls land.
        deltaP = wpsum.tile([128, 9, C], f32)
        for t in range(9):
            nc.tensor.matmul(
                deltaP[:, t, :], LD9[:, t, :], LUTn[:], start=True, stop=True
            )

        # wm3 = Wb^T*dm + deltaT*(0.25*dm), built front-half first so the
        # phase-0 comb slots (and with them the first conv matmuls) unblock
        # before the back half of the weight build finishes.
        WbTm = wtmp.tile([128, 9, C], f32)
        wm3 = wtmp.tile([128, 9, C], f32)
        for sl in (slice(0, 3), slice(3, 9)):
            nc.vector.tensor_scalar_mul(WbTm[:, sl, :], WbTS[:, sl, :], dmb[:, 0:1])
            nc.vector.scalar_tensor_tensor(
                wm3[:, sl, :], deltaP[:, sl, :], dmb[:, 2:3], WbTm[:, sl, :],
                op0=ALU.mult, op1=ALU.add,
            )

        # 16 combined tap matrices.  Row combos over ki (t = 3*ki + kj):
        #   (di=0, a=0): ki0        (di=0, a=1): ki1+ki2
        #   (di=1, a=0): ki0+ki1    (di=1, a=1): ki2
        # and the same pattern over kj for (dj, b).  Adds run on DVE, the
        # pure copy/rounding slots on ACT, interleaved in PE consumption
        # order (phase 0/1 slots before 2/3).
        def vadd(dst, a0, a1):
            nc.vector.tensor_add(dst, a0, a1)

        def acopy(dst, src):
            nc.scalar.activation(dst, src, AF.Identity)

        vadd(comb[:, _slot(0, 0, 0, 1), :], wm3[:, 1, :], wm3[:, 2, :])
        vadd(comb[:, _slot(0, 0, 1, 0), :], wm3[:, 0, :], wm3[:, 1, :])
        acopy(comb[:, _slot(0, 0, 0, 0), :], wm3[:, 0, :])
        acopy(comb[:, _slot(0, 0, 1, 1), :], wm3[:, 2, :])

        R01 = wtmp.tile([128, 3, C], f32)
        vadd(R01[:], wm3[:, 3:6, :], wm3[:, 6:9, :])
        vadd(comb[:, _slot(0, 1, 0, 1), :], R01[:, 1, :], R01[:, 2, :])
        vadd(comb[:, _slot(0, 1, 1, 0), :], R01[:, 0, :], R01[:, 1, :])
        acopy(comb[:, _slot(0, 1, 0, 0), :], R01[:, 0, :])
        acopy(comb[:, _slot(0, 1, 1, 1), :], R01[:, 2, :])

        R10 = wtmp.tile([128, 3, C], f32)
        vadd(R10[:], wm3[:, 0:3, :], wm3[:, 3:6, :])
        vadd(comb[:, _slot(1, 0, 0, 1), :], R10[:, 1, :], R10[:, 2, :])
        vadd(comb[:, _slot(1, 0, 1, 0), :], R10[:, 0, :], R10[:, 1, :])
        acopy(comb[:, _slot(1, 0, 0, 0), :], R10[:, 0, :])
        acopy(comb[:, _slot(1, 0, 1, 1), :], R10[:, 2, :])

        vadd(comb[:, _slot(1, 1, 0, 1), :], wm3[:, 7, :], wm3[:, 8, :])
        vadd(comb[:, _slot(1, 1, 1, 0), :], wm3[:, 6, :], wm3[:, 7, :])
        acopy(comb[:, _slot(1, 1, 0, 0), :], wm3[:, 6, :])
        acopy(comb[:, _slot(1, 1, 1, 1), :], wm3[:, 8, :])

        # demod inputs: square on DVE (keeps ACT free of table swaps), reduce
        # over taps, round to bf16 for a cheap single-pass partition-sum
        # matmul later.  All of this hides behind block 0's conv matmuls.
        sqT = wtmp.tile([128, 9, C], f32)
        nc.vector.tensor_mul(sqT[:], wm3[:], wm3[:])
        s2f = wtmp.tile([128, C], f32)
        nc.vector.tensor_reduce(
            s2f[:], sqT.rearrange("p t o -> p o t"), axis=AX.X, op=ALU.add
        )
        nc.vector.tensor_copy(s2b[:], s2f[:])

    # ---- main conv loop ----
    mpsum = ctx.enter_context(tc.tile_pool(name="mpsum", bufs=8, space="PSUM"))
    opool = ctx.enter_context(tc.tile_pool(name="obuf", bufs=3))

    def block_mms(i0, R, bb):
        lo = BAND_ROWS * bb - 1
        bt = band_tiles[bb]
        ph = []
        for p in range(4):
            di, dj = p >> 1, p & 1
            pt = mpsum.tile([128, R * W], f32, tag="ph", name=f"ph{p}_{i0}")
            for q in range(4):
                a, b = q >> 1, q & 1
                r0 = i0 + (a + di - 1) - lo          # tile row of first x row
                co = b + dj - 1
                rhs = bt[:, r0 : r0 + R, co + 1 : co + 1 + W]
                nc.tensor.matmul(
                    pt[:], comb[:, _slot(di, a, dj, b), :], rhs,
                    start=(q == 0), stop=(q == 3),
                )
            ph.append(pt)
        return ph

    def block_evict(i0, R, ph, split_dma=False):
        # phase-separated output: scale by demod, add bias, round to bf16.
        # dj=0 phases on DVE, dj=1 on ACT; all writes contiguous.  The last
        # block's DMA goes out per-phase so the kernel tail isn't gated on
        # the full 4-phase buffer.
        ob = opool.tile([128, 4, R * W], bf16, tag="ob", name=f"ob_{i0}")
        off = 4 * W * i0
        for p in range(4):
            if p & 1 == 0:
                nc.vector.tensor_scalar(
                    ob[:, p, :], ph[p][:], demP[:, 0:1], dmb[:, 1:2],
                    op0=ALU.mult, op1=ALU.add,
                )
            else:
                nc.scalar.activation(
                    ob[:, p, :], ph[p][:], AF.Identity,
                    bias=dmb[:, 1:2], scale=demP[:, 0:1],
                )
            if split_dma:
                nc.sync.dma_start(
                    y[:, off + p * R * W : off + (p + 1) * R * W], ob[:, p, :]
                )
        if not split_dma:
            nc.sync.dma_start(y[:, off : off + 4 * R * W], ob[:])

    # block 0's matmuls go ahead of the demod partition-sum so the in-order
    # PE never idles waiting for the DVE-side demod inputs; demP lands while
    # block 0/1 stream, just in time for the first eviction.
    ph0 = block_mms(*BLOCKS[0])
    sP = mpsum.tile([128, 1], f32, tag="ph", name="sP")
    nc.tensor.matmul(sP[:], s2b[:], onesS[:], start=True, stop=True)
    nc.vector.tensor_scalar_add(t1[:], sP[:], EPS)
    nc.scalar.sqrt(sdem[:], t1[:])
    nc.vector.reciprocal(demP[:], sdem[:])
    block_evict(BLOCKS[0][0], BLOCKS[0][1], ph0)

    for i0, R, bb in BLOCKS[1:]:
        ph = block_mms(i0, R, bb)
        block_evict(i0, R, ph, split_dma=(i0 == BLOCKS[-1][0]))


def _build():
    nc = bacc.Bacc(
        "TRN2",
        target_bir_lowering=False,
        debug=False,
        enable_asserts=False,
        num_devices=NCORES,
    )
    x = nc.dram_tensor(
        "xb", [C, NBANDS, BAND_TROWS, WP], bf16, kind="ExternalInput"
    ).ap()
    dmbias = nc.dram_tensor("dmbias", [2, C], f32, kind="ExternalInput").ap()
    wbT = nc.dram_tensor("WbT", [C, 9 * C], bf16, kind="ExternalInput").ap()
    lpkD = nc.dram_tensor("lora_pk", [RANK, 10 * C], bf16, kind="ExternalInput").ap()
    ident2 = nc.dram_tensor("ident2", [2, 2], f32, kind="ExternalInput").ap()
    y = nc.dram_tensor("yp", [C, 4 * H * W], bf16, kind="ExternalOutput").ap()

    with tile.TileContext(nc) as tc:
        with ExitStack() as ctx:
            _conv_kernel(ctx, tc, y, x, dmbias, wbT, lpkD, ident2)
    nc.compile()
    return nc


_CACHE = {}


def _get_nc():
    if "nc" not in _CACHE:
        _CACHE["nc"] = _build()
    return _CACHE["nc"]


def _make_in_maps(x, de_mod, Wb, lora_up, lora_down, bias):
    x16 = np.asarray(x, dtype=np.float32).astype(np_bf16)
    de_mod = np.asarray(de_mod, dtype=np.float32)
    Wb = np.asarray(Wb, dtype=np.float32)
    lora_up = np.asarray(lora_up, dtype=np.float32)
    lora_down = np.asarray(lora_down, dtype=np.float32)
    # layout-only host prep: [O,I,3,3] -> [i, (t o)], [R,C,3,3] -> [r, (t i)]
    wbT = np.ascontiguousarray(Wb.transpose(1, 2, 3, 0).reshape(C, 9 * C)).astype(np_bf16)
    lpk = np.concatenate(
        [
            lora_up.T.reshape(RANK, 1, C),
            lora_down.transpose(0, 2, 3, 1).reshape(RANK, 9, C),
        ],
        axis=1,
    ).reshape(RANK, 10 * C).astype(np_bf16)
    bias = np.asarray(bias, dtype=np.float32).reshape(C)
    id2 = np.eye(2, dtype=np.float32)
    # pre-haloed, zero-padded bands: [C, NBANDS, 29, 162]
    xb = np.zeros((B, C, NBANDS, BAND_TROWS, WP), dtype=np_bf16)
    for bb in range(NBANDS):
        lo = BAND_ROWS * bb - 1
        r0, r1 = max(0, lo), min(H - 1, lo + BAND_TROWS - 1)
        xb[:, :, bb, r0 - lo : r1 - lo + 1, 1 : 1 + W] = x16[:, :, r0 : r1 + 1, :]
    in_maps = []
    for b in range(NCORES):
        in_maps.append(
            {
                "xb": np.ascontiguousarray(xb[b]),
                "dmbias": np.ascontiguousarray(np.stack([de_mod[b], bias])),
                "WbT": wbT,
                "lora_pk": np.ascontiguousarray(lpk),
                "ident2": id2,
            }
        )
    return in_maps


def _unshard(yp):
    """[NCORES][C, 4*H*W] bf16 phase-blocks -> [B, C, 2H, 2W] f32."""
    n3 = sum(1 for _, R, _ in BLOCKS if R == 3)     # leading R=3 blocks
    h3 = 3 * n3
    split = 4 * W * h3
    y = np.empty((B, C, 2 * H, 2 * W), np.float32)
    for b in range(NCORES):
        out5 = y[b].reshape(C, H, 2, W, 2)          # [c, i, di, j, dj]
        g1 = np.asarray(yp[b][:, :split], np.float32).reshape(C, n3, 2, 2, 3, W)
        out5[:, :h3] = g1.transpose(0, 1, 4, 2, 5, 3).reshape(C, h3, 2, W, 2)
        if h3 < H:
            g2 = np.asarray(yp[b][:, split:], np.float32).reshape(
                C, -1, 2, 2, 2, W
            )
            out5[:, h3:] = g2.transpose(0, 1, 4, 2, 5, 3).reshape(C, H - h3, 2, W, 2)
    return y


def run(inputs, trace=False, trace_kwargs=None):
    nc = _get_nc()
    in_maps = _make_in_maps(**inputs)
    res = run_bass_kernel_spmd(
        nc,
        in_maps,
        core_ids=list(range(NCORES)),
        trace=trace,
        **(trace_kwargs or {}),
    )
    y = _unshard([res.results[b]["yp"] for b in range(NCORES)])
    return y, res


def kernel(**inputs):
    y, _ = run(inputs)
    return y


# revision 15
# speedup vs baseline: 1.2000x; 1.2000x over previous
"""Trainium2 Bass kernel for nn_NeuronS3DiffUpsample2D.

Reference computation (per sample b):
    up   = nearest-2x-upsample(x[b])                       # [C, 320, 320]
    w    = Wb + 0.25 * einsum('or,rikl->oikl', lora_up, lora_down)
    w_b  = w * de_mod[b, None, :, None, None]              # modulate input chans
    dem  = rsqrt(sum_{i,k,l} w_b^2 + eps)                  # per output chan
    y[b] = conv2d(up, w_b * dem, SAME) + bias

Key algebraic transform: a 3x3 SAME conv on a 2x nearest-upsampled image
decomposes into 4 output phases (di, dj in {0,1}), each a 2x2 conv on the
ORIGINAL 160x160 input:
    y[2i+di, 2j+dj] = sum_{a,b in {0,1}} K[di,dj,a,b] @ x[i+a+di-1, j+b+dj-1]
where the 16 [O, I] matrices K are sums of 1/2/4 of the 9 taps of w.
This is 4/9 of the naive FLOPs and never materializes the upsampled image.

Since the demod scale is per output channel and conv is linear in w, the conv
OUTPUT is scaled by dem[o] (per-partition scalar) at PSUM eviction, fused with
the bias add; weights are only modulated by de_mod on the input-channel axis.

Sharding: data-parallel over batch B=8 across 8 NeuronCores; each core builds
its own per-sample weights locally (replicated W/lora are tiny).  Host-side
work is layout only: per-sample slicing, weight transposition, bf16 rounding,
pre-haloed banding of x, and the final phase interleave of the output.

Everything on the wire is bf16 (error budget is 2e-2; bf16 end-to-end lands
around 3e-3): x is DMA'd as pre-haloed zero-padded bands (one contiguous
descriptor per partition), weights/comb are bf16, and the output is written
phase-separated bf16 (contiguous evictions, half the HBM writes); the host
interleaves the 4 phases into y and upcasts to f32 - pure layout.  PSUM
accumulation and the demod chain stay f32.

Scheduling: the PE is in-order, so the startup is arranged to unblock the
conv stream as early as possible: weight DMAs are queued before the band
DMAs, band 0 is split so its first half lands sooner, the modulated weight
build is split so phase-0 slots exist before the rest, the 8 copy-slots run
on ACT while the 8 add-slots run on DVE, and the demod partition-sum matmul
is emitted AFTER block 0 so it cannot stall the conv stream (its DVE/ACT
inputs compute in the shadow of block 0's matmuls).
"""

import sys
import numpy as np
from contextlib import ExitStack

try:
    import concourse.bass as bass
except ImportError:  # grading env without the axon PYTHONPATH
    sys.path.insert(0, "/opt/trn_rl_repo")
    import concourse.bass as bass
import ml_dtypes
import concourse.tile as tile
from concourse import bacc, mybir
from concourse.bass_utils import run_bass_kernel_spmd

B, C, H, W = 8, 128, 160, 160
RANK = 32
SCALING = 0.25
EPS = 1e-8
WP = W + 2          # padded row length (zero col on each side)
BAND_ROWS = 27      # x-rows per band
NBANDS = (H + BAND_ROWS - 1) // BAND_ROWS   # 6
BAND_TROWS = BAND_ROWS + 2           # band rows incl. halo (29)
B0SPLIT = 14        # band0 lands as rows [0,14) then [14,29)
NCORES = 8

f32 = mybir.dt.float32
bf16 = mybir.dt.bfloat16
np_bf16 = ml_dtypes.bfloat16


def _blocks():
    """(i0, R, band) row blocks: 3 rows per matmul (N=480), except the last
    band's tail 4 rows split 2+2 so every matmul keeps free dim >= 256."""
    out = []
    for bb in range(NBANDS):
        lo, hi = BAND_ROWS * bb, min(BAND_ROWS * (bb + 1), H)
        i = lo
        while i < hi:
            r = 2 if hi - i == 4 else min(3, hi - i)
            out.append((i, r, bb))
            i += r
    return out

BLOCKS = _blocks()


def _slot(di, a, dj, b):
    return 8 * di + 2 * a + 4 * dj + b


def _conv_kernel(ctx, tc, y, x, dmbias, wbT, lpkD, ident2):
    nc = tc.nc
    AF = mybir.ActivationFunctionType
    ALU = mybir.AluOpType
    AX = mybir.AxisListType

    const = ctx.enter_context(tc.tile_pool(name="const", bufs=1))
    bands = ctx.enter_context(tc.tile_pool(name="bands", bufs=3))

    comb = const.tile([128, 16, C], bf16)        # 16 combined taps, [i, slot, o]
    demP = const.tile([128, 1], f32)             # rsqrt demod, per output chan
    dmb = const.tile([128, 3], f32)              # de_mod[i], bias[o], 0.25*de_mod
    s2b = const.tile([128, C], bf16)             # per-(i,o) weight square sums
    onesS = const.tile([128, 1], bf16)
    t1 = const.tile([128, 1], f32)               # sum w^2 + eps
    sdem = const.tile([128, 1], f32)             # sqrt(sum w^2 + eps)

    # de_mod/bias arrive as a [2,128] row pair (single-descriptor DMA) and are
    # PE-transposed onto partitions.
    dmbR = const.tile([2, C], f32)
    nc.sync.dma_start(dmbR[:], dmbias[:])
    id2 = const.tile([2, 2], f32)
    nc.sync.dma_start(id2[:], ident2[:])

    # PE warm-up: the HAM clock gate keeps the PE at 1.2 GHz until it has
    # seen ~3.4us of sustained matmul activity.  A burst of throwaway
    # matmuls on zeroed SBUF during the (otherwise idle) DMA window flips
    # the gate to 2.4 GHz right as the real stream begins.
    wz = const.tile([128, 512], bf16)
    nc.vector.memset(wz[:], 0.0)

    with tc.tile_pool(name="wtmp", bufs=1) as wtmp, tc.tile_pool(
        name="wpsum", bufs=1, space="PSUM"
    ) as wpsum:
        # ---- input DMAs, all on one SWDGE queue so FIFO order gives the
        # tiny weight tensors strict priority over the 7 MB of band traffic
        # (the SDMA engines round-robin *between* queues at packet
        # granularity, which would starve the weights for ~10us).  Band 0
        # is interleaved right after the lora weights and split in two so
        # the first conv matmuls start as early as possible.
        lpk = wtmp.tile([RANK, 10, C], bf16)     # [lora_up^T | lora_down^T]
        nc.gpsimd.dma_start(lpk[:], lpkD[:])
        LUTn = lpk[:, 0, :]                      # lora_up^T: [r, o]
        LD9 = lpk[:, 1:10, :]                    # lora_down^T: [r, t, i]
        WbTS = wtmp.tile([128, 9, C], bf16)      # Wb^T: [i, t, o]
        band_tiles = [
            bands.tile([128, BAND_TROWS, WP], bf16, tag="band", name=f"band{bb}")
            for bb in range(NBANDS)
        ]
        # Interleave the splits so each startup dependency lands as late as
        # it is needed and no earlier-needed transfer queues behind a bigger
        # one: Wb taps 0-2 feed the phase-0 slot chain (~1.9us of DVE work),
        # band rows 0-4 feed only block 0, the rest follows.
        nc.gpsimd.dma_start(WbTS[:, 0:3, :], wbT[:, 0 : 3 * C])
        nc.gpsimd.dma_start(band_tiles[0][:, 0:5, :], x[:, 0, 0:5, :])
        nc.gpsimd.dma_start(WbTS[:, 3:9, :], wbT[:, 3 * C :])
        nc.gpsimd.dma_start(band_tiles[0][:, 5:B0SPLIT, :], x[:, 0, 5:B0SPLIT, :])
        nc.gpsimd.dma_start(band_tiles[0][:, B0SPLIT:, :], x[:, 0, B0SPLIT:, :])
        for bb in range(1, NBANDS):
            nc.gpsimd.dma_start(band_tiles[bb][:], x[:, bb])

        # 12 x N=512 fills ~4.5us: flips the HAM gate AND spans the input
        # DMAs' ~2us completion latency so the PE never idles before the
        # delta matmuls.
        zpsum = wpsum.tile([128, 512], f32)
        for _ in range(12):
            nc.tensor.matmul(zpsum[:], wz[:, 0:128], wz[:], start=True, stop=True)

        dmbP = wpsum.tile([128, 2], f32)
        nc.tensor.transpose(dmbP[:], dmbR[:], id2[:])
        nc.vector.tensor_copy(dmb[:, 0:2], dmbP[:])
        nc.vector.tensor_scalar_mul(dmb[:, 2:3], dmb[:, 0:1], SCALING)
        nc.vector.memset(onesS[:], 1.0)

        # deltaT_unscaled[i, t, o] = sum_r down[r,i,t] * up[o,r]; t=0..2 first
        # so the wm3 front half can start after only 3 matmuls land.
        deltaP = wpsum.tile([128, 9, C], f32)
        for t in range(9):
            nc.tensor.matmul(
                deltaP[:, t, :], LD9[:, t, :], LUTn[:], start=True, stop=True
            )

        # wm3 = Wb^T*dm + deltaT*(0.25*dm), built front-half first so the
        # phase-0 comb slots (and with them the first conv matmuls) unblock
        # before the back half of the weight build finishes.
        WbTm = wtmp.tile([128, 9, C], f32)
        wm3 = wtmp.tile([128, 9, C], f32)
        for sl in (slice(0, 3), slice(3, 9)):
            nc.vector.tensor_scalar_mul(WbTm[:, sl, :], WbTS[:, sl, :], dmb[:, 0:1])
            nc.vector.scalar_tensor_tensor(
                wm3[:, sl, :], deltaP[:, sl, :], dmb[:, 2:3], WbTm[:, sl, :],
                op0=ALU.mult, op1=ALU.add,
            )

        # 16 combined tap matrices.  Row combos over ki (t = 3*ki + kj):
        #   (di=0, a=0): ki0        (di=0, a=1): ki1+ki2
        #   (di=1, a=0): ki0+ki1    (di=1, a=1): ki2
        # and the same pattern over kj for (dj, b).  Adds run on DVE, the
        # pure copy/rounding slots on ACT, interleaved in PE consumption
        # order (phase 0/1 slots before 2/3).
        def vadd(dst, a0, a1):
            nc.vector.tensor_add(dst, a0, a1)

        def acopy(dst, src):
            nc.scalar.activation(dst, src, AF.Identity)

        vadd(comb[:, _slot(0, 0, 0, 1), :], wm3[:, 1, :], wm3[:, 2, :])
        vadd(comb[:, _slot(0, 0, 1, 0), :], wm3[:, 0, :], wm3[:, 1, :])
        acopy(comb[:, _slot(0, 0, 0, 0), :], wm3[:, 0, :])
        acopy(comb[:, _slot(0, 0, 1, 1), :], wm3[:, 2, :])

        R01 = wtmp.tile([128, 3, C], f32)
        vadd(R01[:], wm3[:, 3:6, :], wm3[:, 6:9, :])
        vadd(comb[:, _slot(0, 1, 0, 1), :], R01[:, 1, :], R01[:, 2, :])
        vadd(comb[:, _slot(0, 1, 1, 0), :], R01[:, 0, :], R01[:, 1, :])
        acopy(comb[:, _slot(0, 1, 0, 0), :], R01[:, 0, :])
        acopy(comb[:, _slot(0, 1, 1, 1), :], R01[:, 2, :])

        R10 = wtmp.tile([128, 3, C], f32)
        vadd(R10[:], wm3[:, 0:3, :], wm3[:, 3:6, :])
        vadd(comb[:, _slot(1, 0, 0, 1), :], R10[:, 1, :], R10[:, 2, :])
        vadd(comb[:, _slot(1, 0, 1, 0), :], R10[:, 0, :], R10[:, 1, :])
        acopy(comb[:, _slot(1, 0, 0, 0), :], R10[:, 0, :])
        acopy(comb[:, _slot(1, 0, 1, 1), :], R10[:, 2, :])

        vadd(comb[:, _slot(1, 1, 0, 1), :], wm3[:, 7, :], wm3[:, 8, :])
        vadd(comb[:, _slot(1, 1, 1, 0), :], wm3[:, 6, :], wm3[:, 7, :])
        acopy(comb[:, _slot(1, 1, 0, 0), :], wm3[:, 6, :])
        acopy(comb[:, _slot(1, 1, 1, 1), :], wm3[:, 8, :])

        # demod inputs: square on DVE (keeps ACT free of table swaps), reduce
        # over taps, round to bf16 for a cheap single-pass partition-sum
        # matmul later.  All of this hides behind block 0's conv matmuls.
        sqT = wtmp.tile([128, 9, C], f32)
        nc.vector.tensor_mul(sqT[:], wm3[:], wm3[:])
        s2f = wtmp.tile([128, C], f32)
        nc.vector.tensor_reduce(
            s2f[:], sqT.rearrange("p t o -> p o t"), axis=AX.X, op=ALU.add
        )
        nc.vector.tensor_copy(s2b[:], s2f[:])

    # ---- main conv loop ----
    mpsum = ctx.enter_context(tc.tile_pool(name="mpsum", bufs=8, space="PSUM"))
    opool = ctx.enter_context(tc.tile_pool(name="obuf", bufs=3))

    def block_mms(i0, R, bb):
        lo = BAND_ROWS * bb - 1
        bt = band_tiles[bb]
        ph = []
        for p in range(4):
            di, dj = p >> 1, p & 1
            pt = mpsum.tile([128, R * W], f32, tag="ph", name=f"ph{p}_{i0}")
            for q in range(4):
                a, b = q >> 1, q & 1
                r0 = i0 + (a + di - 1) - lo          # tile row of first x row
                co = b + dj - 1
                rhs = bt[:, r0 : r0 + R, co + 1 : co + 1 + W]
                nc.tensor.matmul(
                    pt[:], comb[:, _slot(di, a, dj, b), :], rhs,
                    start=(q == 0), stop=(q == 3),
                )
            ph.append(pt)
        return ph

    def block_evict(i0, R, ph, split_dma=False):
        # phase-separated output: scale by demod, add bias, round to bf16.
        # dj=0 phases on DVE, dj=1 on ACT; all writes contiguous.  The last
        # block's DMA goes out per-phase so the kernel tail isn't gated on
        # the full 4-phase buffer.
        ob = opool.tile([128, 4, R * W], bf16, tag="ob", name=f"ob_{i0}")
        off = 4 * W * i0
        for p in range(4):
            if p & 1 == 0:
                nc.vector.tensor_scalar(
                    ob[:, p, :], ph[p][:], demP[:, 0:1], dmb[:, 1:2],
                    op0=ALU.mult, op1=ALU.add,
                )
            else:
                nc.scalar.activation(
                    ob[:, p, :], ph[p][:], AF.Identity,
                    bias=dmb[:, 1:2], scale=demP[:, 0:1],
                )
            if split_dma:
                nc.sync.dma_start(
                    y[:, off + p * R * W : off + (p + 1) * R * W], ob[:, p, :]
                )
        if not split_dma:
            nc.sync.dma_start(y[:, off : off + 4 * R * W], ob[:])

    # block 0's matmuls go ahead of the demod partition-sum so the in-order
    # PE never idles waiting for the DVE-side demod inputs; demP lands while
    # block 0/1 stream, just in time for the first eviction.
    ph0 = block_mms(*BLOCKS[0])
    sP = mpsum.tile([128, 1], f32, tag="ph", name="sP")
    nc.tensor.matmul(sP[:], s2b[:], onesS[:], start=True, stop=True)
    nc.vector.tensor_scalar_add(t1[:], sP[:], EPS)
    nc.scalar.sqrt(sdem[:], t1[:])
    nc.vector.reciprocal(demP[:], sdem[:])
    block_evict(BLOCKS[0][0], BLOCKS[0][1], ph0)

    for i0, R, bb in BLOCKS[1:]:
        ph = block_mms(i0, R, bb)
        block_evict(i0, R, ph, split_dma=(i0 == BLOCKS[-1][0]))


def _build():
    nc = bacc.Bacc(
        "TRN2",
        target_bir_lowering=False,
        debug=False,
        enable_asserts=False,
        num_devices=NCORES,
    )
    x = nc.dram_tensor(
        "xb", [C, NBANDS, BAND_TROWS, WP], bf16, kind="ExternalInput"
    ).ap()
    dmbias = nc.dram_tensor("dmbias", [2, C], f32, kind="ExternalInput").ap()
    wbT = nc.dram_tensor("WbT", [C, 9 * C], bf16, kind="ExternalInput").ap()
    lpkD = nc.dram_tensor("lora_pk", [RANK, 10 * C], bf16, kind="ExternalInput").ap()
    ident2 = nc.dram_tensor("ident2", [2, 2], f32, kind="ExternalInput").ap()
    y = nc.dram_tensor("yp", [C, 4 * H * W], bf16, kind="ExternalOutput").ap()

    with tile.TileContext(nc) as tc:
        with ExitStack() as ctx:
            _conv_kernel(ctx, tc, y, x, dmbias, wbT, lpkD, ident2)
    nc.compile()
    return nc


_CACHE = {}


def _get_nc():
    if "nc" not in _CACHE:
        _CACHE["nc"] = _build()
    return _CACHE["nc"]


def _make_in_maps(x, de_mod, Wb, lora_up, lora_down, bias):
    x16 = np.asarray(x, dtype=np.float32).astype(np_bf16)
    de_mod = np.asarray(de_mod, dtype=np.float32)
    Wb = np.asarray(Wb, dtype=np.float32)
    lora_up = np.asarray(lora_up, dtype=np.float32)
    lora_down = np.asarray(lora_down, dtype=np.float32)
    # layout-only host prep: [O,I,3,3] -> [i, (t o)], [R,C,3,3] -> [r, (t i)]
    wbT = np.ascontiguousarray(Wb.transpose(1, 2, 3, 0).reshape(C, 9 * C)).astype(np_bf16)
    lpk = np.concatenate(
        [
            lora_up.T.reshape(RANK, 1, C),
            lora_down.transpose(0, 2, 3, 1).reshape(RANK, 9, C),
        ],
        axis=1,
    ).reshape(RANK, 10 * C).astype(np_bf16)
    bias = np.asarray(bias, dtype=np.float32).reshape(C)
    id2 = np.eye(2, dtype=np.float32)
    # pre-haloed, zero-padded bands: [C, NBANDS, 29, 162]
    xb = np.zeros((B, C, NBANDS, BAND_TROWS, WP), dtype=np_bf16)
    for bb in range(NBANDS):
        lo = BAND_ROWS * bb - 1
        r0, r1 = max(0, lo), min(H - 1, lo + BAND_TROWS - 1)
        xb[:, :, bb, r0 - lo : r1 - lo + 1, 1 : 1 + W] = x16[:, :, r0 : r1 + 1, :]
    in_maps = []
    for b in range(NCORES):
        in_maps.append(
            {
                "xb": np.ascontiguousarray(xb[b]),
                "dmbias": np.ascontiguousarray(np.stack([de_mod[b], bias])),
                "WbT": wbT,
                "lora_pk": np.ascontiguousarray(lpk),
                "ident2": id2,
            }
        )
    return in_maps


def _unshard(yp):
    """[NCORES][C, 4*H*W] bf16 phase-blocks -> [B, C, 2H, 2W] f32."""
    n3 = sum(1 for _, R, _ in BLOCKS if R == 3)     # leading R=3 blocks
    h3 = 3 * n3
    split = 4 * W * h3
    y = np.empty((B, C, 2 * H, 2 * W), np.float32)
    for b in range(NCORES):
        out5 = y[b].reshape(C, H, 2, W, 2)          # [c, i, di, j, dj]
        g1 = np.asarray(yp[b][:, :split], np.float32).reshape(C, n3, 2, 2, 3, W)
        out5[:, :h3] = g1.transpose(0, 1, 4, 2, 5, 3).reshape(C, h3, 2, W, 2)
        if h3 < H:
            g2 = np.asarray(yp[b][:, split:], np.float32).reshape(
                C, -1, 2, 2, 2, W
            )
            out5[:, h3:] = g2.transpose(0, 1, 4, 2, 5, 3).reshape(C, H - h3, 2, W, 2)
    return y


def run(inputs, trace=False, trace_kwargs=None):
    nc = _get_nc()
    in_maps = _make_in_maps(**inputs)
    res = run_bass_kernel_spmd(
        nc,
        in_maps,
        core_ids=list(range(NCORES)),
        trace=trace,
        **(trace_kwargs or {}),
    )
    y = _unshard([res.results[b]["yp"] for b in range(NCORES)])
    return y, res


def kernel(**inputs):
    y, _ = run(inputs)
    return y
